# revision 1
# baseline (speedup 1.0000x reference)
"""MoE top-2 routing kernel for 8 Trainium2 NeuronCores (I-split balanced).

Strategy (expert-parallel + intermediate-dim split for load balance):
  - Host computes the gate: softmax -> top-2 -> renormalized scores.
  - The expert MLP  y = gelu(x @ w1 + b1) @ w2  decomposes exactly over
    halves of the intermediate dim I:  y = sum_h gelu(x @ w1[:,hI] + b1[hI]) @ w2[hI,:].
  - 16 jobs = (expert, I-half). Each core runs 2 jobs: one half of a
    heavy expert (group A = 4 largest token counts) and one half of a
    light expert (group B), so per-core work ~= the mean token count
    instead of the max. The two partial y's per expert are summed on the
    host (+ b2, combine scores, scatter-add).
  - All 4 half-weight matrices are bf16 and SBUF-resident (64 KB/part);
    steady-state DMA is just the token stream.
"""

import sys

sys.path.insert(0, "/opt/trn_rl_repo")

from contextlib import ExitStack

import numpy as np

from concourse import bacc, mybir, tile
from concourse.bass_utils import run_bass_kernel_spmd

E, H, I = 8, 1024, 4096
TOP_K = 2
N_CORES = 8

SUB = 512  # token tile = matmul moving dim = one fp32 PSUM bank
IH = I // 2  # 2048, the I-half
MCH = IH // 128  # 16 output chunks for fc1-half / contraction chunks for fc2

F32 = mybir.dt.float32
BF16 = mybir.dt.bfloat16
BF16_NP = mybir.dt.np(mybir.dt.bfloat16)


def _chunks(C):
    n_full, tail = divmod(C, SUB)
    return [SUB] * n_full + ([tail] if tail else [])


def _build_nc(C, act_func=None, repeat=1):
    """One SPMD program: two half-MLP jobs (A, B) over C=(C_A, C_B) tokens."""
    C_A, C_B = C
    nc = bacc.Bacc(
        "TRN2", target_bir_lowering=False, debug=False, num_devices=N_CORES
    )
    gelu = act_func if act_func is not None else mybir.ActivationFunctionType.Gelu

    dram = {}
    for j, Cj in (("a", C_A), ("b", C_B)):
        dram[f"x{j}"] = nc.dram_tensor(f"x{j}", [128, 8, Cj], BF16,
                                       kind="ExternalInput").ap()
        dram[f"w1{j}"] = nc.dram_tensor(f"w1{j}", [128, MCH, 8, 128], BF16,
                                        kind="ExternalInput").ap()
        dram[f"w2{j}"] = nc.dram_tensor(f"w2{j}", [128, 8, MCH, 128], BF16,
                                        kind="ExternalInput").ap()
        dram[f"b1{j}"] = nc.dram_tensor(f"b1{j}", [128, MCH], F32,
                                        kind="ExternalInput").ap()
        nch = len(_chunks(Cj))
        dram[f"y{j}"] = nc.dram_tensor(f"y{j}", [nch, 8, 128, SUB], F32,
                                       kind="ExternalOutput").ap()

    with tile.TileContext(nc) as tc, ExitStack() as ctx:
        wpool = ctx.enter_context(tc.tile_pool(name="w", bufs=1))
        bpool = ctx.enter_context(tc.tile_pool(name="b", bufs=1))
        xpool = ctx.enter_context(tc.tile_pool(name="x", bufs=2))
        hpool = ctx.enter_context(tc.tile_pool(name="h", bufs=1))
        ypool = ctx.enter_context(tc.tile_pool(name="y", bufs=3))
        ps1 = ctx.enter_context(tc.tile_pool(name="ps1", bufs=3, space="PSUM"))
        ps2 = ctx.enter_context(tc.tile_pool(name="ps2", bufs=3, space="PSUM"))

        res = {}
        for j in ("a", "b"):
            b1t = bpool.tile([128, MCH], F32, name=f"b1t{j}")
            nc.sync.dma_start(out=b1t[:], in_=dram[f"b1{j}"])
            w1t = wpool.tile([128, MCH, 8, 128], BF16, name=f"w1t{j}")
            w2t = wpool.tile([128, 8, MCH, 128], BF16, name=f"w2t{j}")
            for s in range(4):
                nc.gpsimd.dma_start(
                    out=w1t[:, s * 4 : (s + 1) * 4],
                    in_=dram[f"w1{j}"][:, s * 4 : (s + 1) * 4],
                )
            for s in range(4):
                nc.gpsimd.dma_start(
                    out=w2t[:, s * 2 : (s + 1) * 2],
                    in_=dram[f"w2{j}"][:, s * 2 : (s + 1) * 2],
                )
            res[j] = (b1t, w1t, w2t)

        rep_ctx = tc.For_i(0, repeat, 1) if repeat > 1 else None
        if rep_ctx is not None:
            ctx.enter_context(rep_ctx)

        for j, Cj in (("a", C_A), ("b", C_B)):
            b1t, w1t, w2t = res[j]
            xd, yd = dram[f"x{j}"], dram[f"y{j}"]
            for t, N in enumerate(_chunks(Cj)):
                t0 = t * SUB
                xt = xpool.tile([128, 8, N], BF16, name="xt")
                nc.sync.dma_start(out=xt[:], in_=xd[:, :, t0 : t0 + N])

                ht = hpool.tile([128, MCH, N], BF16, name="ht")

                # fc1 half + gelu: MCH output chunks, contraction over 8 k-chunks
                for mc in range(MCH):
                    ps = ps1.tile([128, N], F32, name="psa")
                    for kc in range(8):
                        nc.tensor.matmul(
                            ps[:],
                            lhsT=w1t[:, mc, kc, :],
                            rhs=xt[:, kc, :],
                            start=(kc == 0),
                            stop=(kc == 7),
                        )
                    nc.scalar.activation(
                        out=ht[:, mc, :],
                        in_=ps[:],
                        func=gelu,
                        bias=b1t[:, mc : mc + 1],
                        scale=1.0,
                    )

                # fc2 half: 8 output chunks, contraction over MCH i-chunks
                for m2 in range(8):
                    ps = ps2.tile([128, N], F32, name="psb")
                    for ic in range(MCH):
                        nc.tensor.matmul(
                            ps[:],
                            lhsT=w2t[:, m2, ic, :],
                            rhs=ht[:, ic, :],
                            start=(ic == 0),
                            stop=(ic == MCH - 1),
                        )
                    yt = ypool.tile([128, N], F32, name="yt")
                    nc.vector.tensor_copy(out=yt[:], in_=ps[:])
                    nc.sync.dma_start(out=yd[t, m2, :, 0:N], in_=yt[:])

    nc.compile()
    return nc


def _route(x_flat, gate_w, gate_b, alpha):
    """Host gate: returns per-expert (row_indices, combine_scores)."""
    logits = x_flat @ gate_w + gate_b
    m = logits.max(axis=-1, keepdims=True)
    p = np.exp(logits - m)
    p /= p.sum(axis=-1, keepdims=True)
    idx = np.argpartition(p, E - TOP_K, axis=-1)[:, -TOP_K:]  # top-2 (unordered)
    vals = np.take_along_axis(p, idx, axis=-1)
    sc = vals / vals.sum(axis=-1, keepdims=True)
    sc = sc * alpha[idx]
    routes = []
    for e in range(E):
        mask = idx == e  # at most one True per row (top-k distinct)
        rows = np.nonzero(mask.any(axis=1))[0]
        scores = sc[mask]  # row-major order matches `rows`
        routes.append((rows, scores.astype(np.float32)))
    return routes


def _pad64(n):
    return max(SUB, ((n + 63) // 64) * 64)


def prepare(hidden_states, gate_w, gate_b, w1, b1, w2, b2, alpha):
    """Host routing + input prep. Returns (nc, in_maps, state)."""
    x = np.asarray(hidden_states, dtype=np.float32)
    gate_w = np.asarray(gate_w, dtype=np.float32)
    gate_b = np.asarray(gate_b, dtype=np.float32)
    w1 = np.asarray(w1, dtype=np.float32)
    b1 = np.asarray(b1, dtype=np.float32)
    w2 = np.asarray(w2, dtype=np.float32)
    b2 = np.asarray(b2, dtype=np.float32)
    alpha = np.asarray(alpha, dtype=np.float32)

    B, S, Hd = x.shape
    T = B * S
    xf = x.reshape(T, Hd)

    routes = _route(xf, gate_w, gate_b, alpha)
    order = sorted(range(E), key=lambda e: -len(routes[e][0]))
    grp_a, grp_b = order[:4], order[4:]
    C_A = _pad64(max(len(routes[e][0]) for e in grp_a))
    C_B = _pad64(max(len(routes[e][0]) for e in grp_b))
    C = (C_A, C_B)

    nc = _build_nc(C)

    def xde(e, Cj):
        rows = routes[e][0]
        out = np.zeros((128, 8, Cj), dtype=BF16_NP)
        if len(rows):
            out[:, :, : len(rows)] = (
                xf[rows].astype(BF16_NP).T.reshape(8, 128, len(rows))
                .transpose(1, 0, 2)
            )
        return out

    def w1de(e, h):
        wh = w1[e][:, h * IH : (h + 1) * IH]  # [1024, 2048]
        return np.ascontiguousarray(
            wh.astype(BF16_NP).reshape(8, 128, MCH, 128).transpose(1, 2, 0, 3)
        )

    def w2de(e, h):
        wh = w2[e][h * IH : (h + 1) * IH, :]  # [2048, 1024]
        return np.ascontiguousarray(
            wh.astype(BF16_NP).reshape(MCH, 128, 8, 128).transpose(1, 2, 0, 3)
        )

    def b1de(e, h):
        return np.ascontiguousarray(
            b1[e][h * IH : (h + 1) * IH].reshape(MCH, 128).T
        )

    in_maps = []
    xa_cache = {e: xde(e, C_A) for e in grp_a}
    xb_cache = {e: xde(e, C_B) for e in grp_b}
    for c in range(N_CORES):
        ea, eb = grp_a[c % 4], grp_b[c % 4]
        ha = 0 if c < 4 else 1  # A-half handled by this core
        hb = 1 if c < 4 else 0  # B-half (opposite arrangement)
        in_maps.append({
            "xa": xa_cache[ea], "w1a": w1de(ea, ha), "w2a": w2de(ea, ha),
            "b1a": b1de(ea, ha),
            "xb": xb_cache[eb], "w1b": w1de(eb, hb), "w2b": w2de(eb, hb),
            "b1b": b1de(eb, hb),
        })

    state = dict(routes=routes, C=C, grp_a=grp_a, grp_b=grp_b,
                 b2=b2, B=B, S=S, Hd=Hd, T=T)
    return nc, in_maps, state


def finalize(results, state):
    routes, b2 = state["routes"], state["b2"]
    C_A, C_B = state["C"]
    T, Hd = state["T"], state["Hd"]
    out = np.zeros((T, Hd), dtype=np.float32)
    for grp, key, Cj in (("grp_a", "ya", C_A), ("grp_b", "yb", C_B)):
        nch = len(_chunks(Cj))
        for i, e in enumerate(state[grp]):
            rows, scores = routes[e]
            if not len(rows):
                continue
            # halves of expert e live on cores i and i+4
            y = (results[i][key] + results[i + 4][key])  # [nch, 8, 128, SUB]
            ye = y.transpose(0, 3, 1, 2).reshape(nch * SUB, Hd)[: len(rows)]
            out[rows] += scores[:, None] * (ye + b2[e])
    return out.reshape(state["B"], state["S"], Hd)


def kernel(hidden_states, gate_w, gate_b, w1, b1, w2, b2, alpha):
    nc, in_maps, state = prepare(
        hidden_states, gate_w, gate_b, w1, b1, w2, b2, alpha
    )
    res = run_bass_kernel_spmd(nc, in_maps, list(range(N_CORES)))
    return finalize(res.results, state)



# revision 2
# speedup vs baseline: 1.0152x; 1.0152x over previous
"""MoE top-2 routing kernel for 8 Trainium2 NeuronCores (expert-eighths).

Strategy (perfectly balanced expert-parallel):
  - Host computes the gate: softmax -> top-2 -> renormalized scores.
  - The expert MLP  y = gelu(x @ w1 + b1) @ w2  decomposes exactly over
    eighths of the intermediate dim I: each core holds I-chunks
    [4c : 4c+4) (of 32) for EVERY expert -> 1/8 of all weights, and
    computes a partial y for every (token, expert) pair.
  - The SPMD program has 8 slots, slot s = expert order[s]; all cores
    process the same expert in a slot, so the slot's token capacity is
    that expert's exact routed count (pad8) -- zero balance padding.
    Per-core streamed matmul columns hit the 64 * sum(counts) floor.
  - Host sums the 8 per-core partials (+ b2, combine scores,
    scatter-add). Partials are emitted in bf16 to halve DMA.
  - All weights are bf16 and SBUF-resident (128 KB/partition); steady
    state DMA is the token stream in and bf16 partials out.
"""

import sys

sys.path.insert(0, "/opt/trn_rl_repo")

from contextlib import ExitStack

import numpy as np

from concourse import bacc, mybir, tile
from concourse.bass_utils import run_bass_kernel_spmd

E, H, I = 8, 1024, 4096
TOP_K = 2
N_CORES = 8

SUB = 512  # token tile = matmul moving dim = one fp32 PSUM bank
IE = I // 8  # 512, the I-eighth per core per expert
MCH = IE // 128  # 4 fc1-output chunks / fc2-contraction chunks per slot

F32 = mybir.dt.float32
BF16 = mybir.dt.bfloat16
BF16_NP = mybir.dt.np(mybir.dt.bfloat16)


def _chunks(C):
    n_full, tail = divmod(C, SUB)
    return [SUB] * n_full + ([tail] if tail else [])


def _build_nc(C, act_func=None, repeat=1):
    """One SPMD program: 8 slots, slot s processes C[s] tokens through
    this core's I-eighth of one expert."""
    nc = bacc.Bacc(
        "TRN2", target_bir_lowering=False, debug=False, num_devices=N_CORES
    )
    gelu = act_func if act_func is not None else mybir.ActivationFunctionType.Gelu

    dram = {}
    for s, Cs in enumerate(C):
        dram[f"x{s}"] = nc.dram_tensor(f"x{s}", [128, 8, Cs], BF16,
                                       kind="ExternalInput").ap()
        dram[f"w1{s}"] = nc.dram_tensor(f"w1{s}", [128, MCH, 8, 128], BF16,
                                        kind="ExternalInput").ap()
        dram[f"w2{s}"] = nc.dram_tensor(f"w2{s}", [128, 8, MCH, 128], BF16,
                                        kind="ExternalInput").ap()
        dram[f"b1{s}"] = nc.dram_tensor(f"b1{s}", [128, MCH], F32,
                                        kind="ExternalInput").ap()
        nch = len(_chunks(Cs))
        dram[f"y{s}"] = nc.dram_tensor(f"y{s}", [nch, 128, 8, SUB], BF16,
                                       kind="ExternalOutput").ap()

    with tile.TileContext(nc) as tc, ExitStack() as ctx:
        wpool = ctx.enter_context(tc.tile_pool(name="w", bufs=1))
        bpool = ctx.enter_context(tc.tile_pool(name="b", bufs=1))
        xpool = ctx.enter_context(tc.tile_pool(name="x", bufs=3))
        hpool = ctx.enter_context(tc.tile_pool(name="h", bufs=2))
        ypool = ctx.enter_context(tc.tile_pool(name="y", bufs=2))
        ps1 = ctx.enter_context(tc.tile_pool(name="ps1", bufs=3, space="PSUM"))
        ps2 = ctx.enter_context(tc.tile_pool(name="ps2", bufs=3, space="PSUM"))

        res = {}
        for s in range(8):
            b1t = bpool.tile([128, MCH], F32, name=f"b1t{s}")
            nc.sync.dma_start(out=b1t[:], in_=dram[f"b1{s}"])
            w1t = wpool.tile([128, MCH, 8, 128], BF16, name=f"w1t{s}")
            w2t = wpool.tile([128, 8, MCH, 128], BF16, name=f"w2t{s}")
            nc.gpsimd.dma_start(out=w1t[:], in_=dram[f"w1{s}"])
            nc.gpsimd.dma_start(out=w2t[:], in_=dram[f"w2{s}"])
            res[s] = (b1t, w1t, w2t)

        rep_ctx = tc.For_i(0, repeat, 1) if repeat > 1 else None
        if rep_ctx is not None:
            ctx.enter_context(rep_ctx)

        for s, Cs in enumerate(C):
            b1t, w1t, w2t = res[s]
            xd, yd = dram[f"x{s}"], dram[f"y{s}"]
            for t, N in enumerate(_chunks(Cs)):
                t0 = t * SUB
                xt = xpool.tile([128, 8, N], BF16, name="xt")
                nc.sync.dma_start(out=xt[:], in_=xd[:, :, t0 : t0 + N])

                ht = hpool.tile([128, MCH, N], BF16, name="ht")

                # fc1 eighth + gelu: MCH output chunks, contraction over 8
                for mc in range(MCH):
                    ps = ps1.tile([128, N], F32, name="psa")
                    for kc in range(8):
                        nc.tensor.matmul(
                            ps[:],
                            lhsT=w1t[:, mc, kc, :],
                            rhs=xt[:, kc, :],
                            start=(kc == 0),
                            stop=(kc == 7),
                        )
                    nc.scalar.activation(
                        out=ht[:, mc, :],
                        in_=ps[:],
                        func=gelu,
                        bias=b1t[:, mc : mc + 1],
                        scale=1.0,
                    )

                # fc2 eighth: 8 output chunks, contraction over MCH i-chunks
                yt = ypool.tile([128, 8, N], BF16, name="yt")
                for m2 in range(8):
                    ps = ps2.tile([128, N], F32, name="psb")
                    for ic in range(MCH):
                        nc.tensor.matmul(
                            ps[:],
                            lhsT=w2t[:, m2, ic, :],
                            rhs=ht[:, ic, :],
                            start=(ic == 0),
                            stop=(ic == MCH - 1),
                        )
                    nc.vector.tensor_copy(out=yt[:, m2, :], in_=ps[:])
                nc.sync.dma_start(out=yd[t, :, :, 0:N], in_=yt[:])

    nc.compile()
    return nc


def _route(x_flat, gate_w, gate_b, alpha):
    """Host gate: returns per-expert (row_indices, combine_scores)."""
    logits = x_flat @ gate_w + gate_b
    m = logits.max(axis=-1, keepdims=True)
    p = np.exp(logits - m)
    p /= p.sum(axis=-1, keepdims=True)
    idx = np.argpartition(p, E - TOP_K, axis=-1)[:, -TOP_K:]  # top-2 (unordered)
    vals = np.take_along_axis(p, idx, axis=-1)
    sc = vals / vals.sum(axis=-1, keepdims=True)
    sc = sc * alpha[idx]
    routes = []
    for e in range(E):
        mask = idx == e  # at most one True per row (top-k distinct)
        rows = np.nonzero(mask.any(axis=1))[0]
        scores = sc[mask]  # row-major order matches `rows`
        routes.append((rows, scores.astype(np.float32)))
    return routes


def _pad8(n):
    return max(8, ((n + 7) // 8) * 8)


def prepare(hidden_states, gate_w, gate_b, w1, b1, w2, b2, alpha):
    """Host routing + input prep. Returns (nc, in_maps, state)."""
    x = np.asarray(hidden_states, dtype=np.float32)
    gate_w = np.asarray(gate_w, dtype=np.float32)
    gate_b = np.asarray(gate_b, dtype=np.float32)
    w1 = np.asarray(w1, dtype=np.float32)
    b1 = np.asarray(b1, dtype=np.float32)
    w2 = np.asarray(w2, dtype=np.float32)
    b2 = np.asarray(b2, dtype=np.float32)
    alpha = np.asarray(alpha, dtype=np.float32)

    B, S, Hd = x.shape
    T = B * S
    xf = x.reshape(T, Hd)

    routes = _route(xf, gate_w, gate_b, alpha)
    order = sorted(range(E), key=lambda e: -len(routes[e][0]))
    C = tuple(_pad8(len(routes[e][0])) for e in order)

    nc = _build_nc(C)

    def xde(e, Cs):
        rows = routes[e][0]
        out = np.zeros((128, 8, Cs), dtype=BF16_NP)
        if len(rows):
            out[:, :, : len(rows)] = (
                xf[rows].astype(BF16_NP).T.reshape(8, 128, len(rows))
                .transpose(1, 0, 2)
            )
        return out

    def w1de(e, c):
        wh = w1[e][:, c * IE : (c + 1) * IE]  # [1024, 512]
        return np.ascontiguousarray(
            wh.astype(BF16_NP).reshape(8, 128, MCH, 128).transpose(1, 2, 0, 3)
        )

    def w2de(e, c):
        wh = w2[e][c * IE : (c + 1) * IE, :]  # [512, 1024]
        return np.ascontiguousarray(
            wh.astype(BF16_NP).reshape(MCH, 128, 8, 128).transpose(1, 2, 0, 3)
        )

    def b1de(e, c):
        return np.ascontiguousarray(
            b1[e][c * IE : (c + 1) * IE].reshape(MCH, 128).T
        )

    x_cache = {s: xde(e, C[s]) for s, e in enumerate(order)}
    in_maps = []
    for c in range(N_CORES):
        m = {}
        for s, e in enumerate(order):
            m[f"x{s}"] = x_cache[s]
            m[f"w1{s}"] = w1de(e, c)
            m[f"w2{s}"] = w2de(e, c)
            m[f"b1{s}"] = b1de(e, c)
        in_maps.append(m)

    state = dict(routes=routes, C=C, order=order,
                 b2=b2, B=B, S=S, Hd=Hd, T=T)
    return nc, in_maps, state


def finalize(results, state):
    routes, b2 = state["routes"], state["b2"]
    C, order = state["C"], state["order"]
    T, Hd = state["T"], state["Hd"]
    out = np.zeros((T, Hd), dtype=np.float32)
    for s, e in enumerate(order):
        rows, scores = routes[e]
        if not len(rows):
            continue
        # partial-sum the 8 cores' I-eighths of expert e
        y = np.zeros(results[0][f"y{s}"].shape, dtype=np.float32)
        for c in range(N_CORES):
            y += results[c][f"y{s}"].astype(np.float32)
        # y: [nch, 128, 8, SUB] -> [tokens, Hd]
        ye = y.transpose(0, 3, 2, 1).reshape(-1, Hd)[: len(rows)]
        out[rows] += scores[:, None] * (ye + b2[e])
    return out.reshape(state["B"], state["S"], Hd)


def kernel(hidden_states, gate_w, gate_b, w1, b1, w2, b2, alpha):
    nc, in_maps, state = prepare(
        hidden_states, gate_w, gate_b, w1, b1, w2, b2, alpha
    )
    res = run_bass_kernel_spmd(nc, in_maps, list(range(N_CORES)))
    return finalize(res.results, state)
